# revision 10
# baseline (speedup 1.0000x reference)
"""Causal GQA self-attention (B=4, T=2048, C=1024, H=16, KV=4) on 8 TRN2
NeuronCores via Bass/Tile.

Sharding: core = (batch b, causal-block pair). T is split into four 512-row
blocks k=0..3 with causal cost ~(k+1); pattern A = blocks {0, 3}, pattern
B = {1, 2} — equal work. 8 cores = 4 batches x {A, B}. Two compiled SPMD
programs (one per pattern); no collectives — each core computes its full
output rows locally.

Per core: K/V/Q projections from host-transposed x^T, flash-style attention
in S^T=[keys, queries] layout (f32r matmuls), softmax denominator via an
appended ones-column in V, exp on ScalarE from PSUM, 1/l via exp(-ln(l)) on
ScalarE, broadcast of 1/l across partitions via a K=1 rank-1 matmul, output
projection with biases as K=1 rank-1 matmuls.
"""
import sys

if '/opt/trn_rl_repo' not in sys.path:
    sys.path.insert(0, '/opt/trn_rl_repo')

import numpy as np

# ---------------------------------------------------------------------------
# TileContext tail-drain patch: this walrus rejects instructions carrying
# more than ~2 sync waits; Tile's kernel-tail SP drain accumulates one wait
# per outstanding semaphore. Split the waits onto individual SP nops.
from concourse import tile as _tile


def _patched_drain_and_barrier(self, tick_clock, wait_clock):
    from concourse.tile import ScopedClock

    drain_inst = self.nc.sync.drain()
    wait_clock.add_sem_waits(
        drain_inst.ins, ScopedClock({None: tick_clock.global_clock})
    )
    si = drain_inst.ins.sync_info
    waits = list(si.on_wait) if si and si.on_wait else []
    if len(waits) > 1:
        drain_inst.ins.sync_info = type(si)(on_wait=[waits[0]], on_update=si.on_update)
        for w in waits[1:]:
            nop = self.nc.sync.nop(nofuse=True)
            nop.ins.sync_info = type(si)(on_wait=[w], on_update=[])

    self.nc.all_engine_barrier()
    assert self.sems is not None
    popped = self.nc._tile_sem_poison_stack.pop()
    assert popped is self._sem_poison
    self.nc.clear_and_free_semaphores(list(self.sems.allocated().values()))
    self.nc.all_engine_barrier()


_tile.TileContext._drain_and_barrier = _patched_drain_and_barrier
# ---------------------------------------------------------------------------

_wsplit_counter = [0]


def split_multi_waits(nc, max_waits=1):
    """Walrus in this env rejects instructions with more than ~1-2 embedded
    sync waits. Hoist extra waits onto same-engine NoOps inserted before."""
    import concourse.mybir as mybir

    for f in nc.m.functions:
        for b in f.blocks:
            il = b.instructions
            i = 0
            while i < len(il):
                inst = il[i]
                si = inst.sync_info
                if si is not None and si.on_wait and len(si.on_wait) > max_waits:
                    waits = list(si.on_wait)
                    inst.sync_info = type(si)(
                        on_wait=waits[-max_waits:], on_update=si.on_update or []
                    )
                    for w in waits[: len(waits) - max_waits]:
                        _wsplit_counter[0] += 1
                        nop = mybir.InstNoOp(
                            name=f"wsplit-{_wsplit_counter[0]}", ins=[], outs=[]
                        )
                        nop.engine = inst.engine
                        nop.sync_info = type(si)(on_wait=[w], on_update=[])
                        il.insert(i, nop)
                        i += 1
                i += 1

import concourse.bass as bass
import concourse.mybir as mybir
from concourse import tile

F32 = mybir.dt.float32
F32R = mybir.dt.float32r
EXPF = mybir.ActivationFunctionType.Exp
LOGF = mybir.ActivationFunctionType.Ln
ADD = mybir.AluOpType.add
MULT = mybir.AluOpType.mult

B = 4
T = 2048
C = 1024
HD = 64
H = 16
KV = 4
NI = C // 128          # 8 contraction i-tiles
W = 512                # t-block width
NEG = -1e30

# head pairs (even-kv head, odd-kv head) sharing a 128-partition QT chunk
PAIRS = [(0, 4), (1, 5), (2, 6), (3, 7), (8, 12), (9, 13), (10, 14), (11, 15)]
HEADPOS = {}
for _m, (_ha, _hb) in enumerate(PAIRS):
    HEADPOS[_ha] = (_m, 0)
    HEADPOS[_hb] = (_m, 64)
PERM_Q = np.concatenate(
    [np.arange(h * HD, (h + 1) * HD) for pair in PAIRS for h in pair]
)

DBG_NO_MASK = False
DBG_NO_NORM = False

VAR_A = (0, 1536)
VAR_B = (512, 1024)


def group_plan(S):
    """Split S s-tiles into groups alternating max sizes 4,2 (PSUM buffers:
    A=4 banks, B=2 banks; 1 bank Y accum, 1 bank 1/l broadcast)."""
    sizes = []
    cap = [4, 2]
    i = 0
    left = S
    while left > 0:
        g = min(cap[i % 2], left)
        sizes.append(g)
        left -= g
        i += 1
    return sizes


def build_program(t0s, T_ctx=T, split_waits=True, upto=4):
    """One SPMD program for cores whose q-blocks start at t0s (globals)."""
    NB = len(t0s)
    TLOC = NB * W
    S_blks = [(t0 + W) // 128 for t0 in t0s]
    NS_KV = max(S_blks)
    NSB4 = (NS_KV * 128 + W - 1) // W

    nc = bass.Bass("TRN2")

    xT = nc.dram_tensor("xT", [C, T_ctx], F32R, kind="ExternalInput")
    wqT = nc.dram_tensor("wqT", [C, C], F32R, kind="ExternalInput")
    wkT = nc.dram_tensor("wkT", [C, KV * HD], F32R, kind="ExternalInput")
    wvT = nc.dram_tensor("wvT", [C, KV * HD], F32R, kind="ExternalInput")
    woT = nc.dram_tensor("woT", [C, C], F32R, kind="ExternalInput")
    bq_col_d = nc.dram_tensor("bq_col", [128, 8], F32, kind="ExternalInput")
    bk_col_d = nc.dram_tensor("bk_col", [128, 2], F32, kind="ExternalInput")
    bv_row_d = nc.dram_tensor("bv_row", [1, KV * HD], F32R, kind="ExternalInput")
    bo_row_d = nc.dram_tensor("bo_row", [1, C], F32R, kind="ExternalInput")
    ones_row_d = nc.dram_tensor("ones_row", [1, W], F32R, kind="ExternalInput")
    maskcat_d = nc.dram_tensor("maskcat", [128, 640], F32, kind="ExternalInput")
    vones_d = nc.dram_tensor("vones", [128, 64], F32R, kind="ExternalInput")
    out = nc.dram_tensor("out", [TLOC, C], F32, kind="ExternalOutput")

    with tile.TileContext(nc) as tc:
        with (
            tc.tile_pool(name="const", bufs=1) as constp,
            tc.tile_pool(name="big", bufs=1) as big,
        ):
            mask_sb = constp.tile([128, 640], F32, tag="mask")
            bq_col = constp.tile([128, 8], F32, tag="bqc")
            bk_col = constp.tile([128, 2], F32, tag="bkc")
            bv_row = constp.tile([1, KV * HD], F32R, tag="bvr")
            bo_row = constp.tile([1, C], F32R, tag="bor")
            ones_row = constp.tile([1, W], F32R, tag="ones")
            nc.sync.dma_start(out=mask_sb[:], in_=maskcat_d[:])
            nc.sync.dma_start(out=bq_col[:], in_=bq_col_d[:])
            nc.sync.dma_start(out=bk_col[:], in_=bk_col_d[:])
            nc.sync.dma_start(out=bv_row[:], in_=bv_row_d[:])
            nc.sync.dma_start(out=bo_row[:], in_=bo_row_d[:])
            nc.sync.dma_start(out=ones_row[:], in_=ones_row_d[:])

            KT_sb = big.tile([128, 2 * T_ctx], F32R, tag="KT")
            V_sb = big.tile([128, 16 * 260], F32R, tag="V")
            QT_sb = big.tile([128, 8 * TLOC], F32R, tag="QT")
            YT_sb = big.tile([128, 8 * TLOC], F32R, tag="YT")

            vview = V_sb[:].rearrange("p (s k e) -> p s k e", s=16, k=KV)
            nc.sync.dma_start(
                out=vview[:, :, :, 64:65],
                in_=vones_d[:].rearrange("p (s k o) -> p s k o", s=16, k=KV),
            )

            # ---------------- Phase 1+2: projections ----------------
            with tc.tile_pool(name="phxt", bufs=1) as phxt:
                xT_sb = phxt.tile([128, NI * T_ctx], F32R, tag="xT")
                for it in range(NI):
                    nc.sync.dma_start(
                        out=xT_sb[:, it * T_ctx : (it + 1) * T_ctx],
                        in_=xT[it * 128 : (it + 1) * 128, :],
                    )

                with (
                    tc.tile_pool(name="phkv", bufs=1) as phkv,
                    tc.tile_pool(name="pskv", bufs=4, space="PSUM") as pskv,
                ):
                    wk_sb = phkv.tile([128, NI * 256], F32R, tag="wk")
                    wv_sb = phkv.tile([128, NI * 256], F32R, tag="wv")
                    for it in range(NI):
                        nc.sync.dma_start(
                            out=wk_sb[:, it * 256 : (it + 1) * 256],
                            in_=wkT[it * 128 : (it + 1) * 128, :],
                        )
                        nc.sync.dma_start(
                            out=wv_sb[:, it * 256 : (it + 1) * 256],
                            in_=wvT[it * 128 : (it + 1) * 128, :],
                        )
                    for j in range(2):
                        for sb in range(NSB4):
                            scols = min(W, NS_KV * 128 - sb * W)
                            ps = pskv.tile([128, W], F32, tag="pk")
                            for it in range(NI):
                                nc.tensor.matmul(
                                    ps[:, :scols],
                                    wk_sb[:, it * 256 + j * 128 : it * 256 + (j + 1) * 128],
                                    xT_sb[:, it * T_ctx + sb * W : it * T_ctx + sb * W + scols],
                                    start=(it == 0),
                                    stop=(it == NI - 1),
                                )
                            nc.vector.tensor_scalar_add(
                                KT_sb[:, j * T_ctx + sb * W : j * T_ctx + sb * W + scols],
                                ps[:, :scols],
                                bk_col[:, j : j + 1],
                            )
                    for st in range(NS_KV):
                        ps = pskv.tile([128, 256], F32, tag="pv")
                        for it in range(NI):
                            nc.tensor.matmul(
                                ps[:],
                                xT_sb[:, it * T_ctx + st * 128 : it * T_ctx + (st + 1) * 128],
                                wv_sb[:, it * 256 : (it + 1) * 256],
                                start=(it == 0),
                                stop=False,
                            )
                        nc.tensor.matmul(
                            ps[:],
                            ones_row[:1, 0:128],
                            bv_row[:1, :],
                            start=False,
                            stop=True,
                        )
                        dst = V_sb[:, st * 260 : st * 260 + 260].rearrange(
                            "p (k e) -> p k e", k=KV
                        )[:, :, 0:64]
                        src = ps[:].rearrange("p (k d) -> p k d", k=KV)
                        nc.vector.tensor_copy(dst, src)

                with (
                    tc.tile_pool(name="phq", bufs=1) as phq,
                    tc.tile_pool(name="psq", bufs=4, space="PSUM") as psq,
                ):
                    wq_sb = phq.tile([128, NI * C], F32R, tag="wq")
                    for it in range(NI):
                        nc.sync.dma_start(
                            out=wq_sb[:, it * C : (it + 1) * C],
                            in_=wqT[it * 128 : (it + 1) * 128, :],
                        )
                    for m in range(8):
                        for tb in range(NB):
                            ps = psq.tile([128, W], F32, tag="pq")
                            for it in range(NI):
                                nc.tensor.matmul(
                                    ps[:],
                                    wq_sb[:, it * C + m * 128 : it * C + (m + 1) * 128],
                                    xT_sb[:, it * T_ctx + t0s[tb] : it * T_ctx + t0s[tb] + W],
                                    start=(it == 0),
                                    stop=(it == NI - 1),
                                )
                            nc.vector.tensor_scalar_add(
                                QT_sb[:, m * TLOC + tb * W : m * TLOC + (tb + 1) * W],
                                ps[:],
                                bq_col[:, m : m + 1],
                            )

            # ---------------- Phase 3: attention ----------------
            if upto < 3:
                return _finish(nc, split_waits)
            with (
                tc.tile_pool(name="ph34", bufs=1) as ph34,
                tc.tile_pool(name="work", bufs=2) as work,
            ):
                wo_sb = ph34.tile([128, NI * C], F32R, tag="wo")
                for ct in range(8):
                    nc.sync.dma_start(
                        out=wo_sb[:, ct * C : (ct + 1) * C],
                        in_=woT[ct * 128 : (ct + 1) * 128, :],
                    )

                with (
                    tc.tile_pool(name="psA", bufs=1, space="PSUM") as psA,
                    tc.tile_pool(name="psB", bufs=1, space="PSUM") as psB,
                    tc.tile_pool(name="psY", bufs=1, space="PSUM") as psY,
                    tc.tile_pool(name="psN", bufs=1, space="PSUM") as psN,
                ):
                    norm_queue = []
                    for tb in range(NB):
                        S_blk = S_blks[tb]
                        sizes = group_plan(S_blk)
                        for kv in range(KV):
                            kbase = (kv % 2) * 64
                            kcol = (kv // 2) * T_ctx
                            for g in range(4):
                                h = kv * 4 + g
                                m, hbase = HEADPOS[h]
                                qlo = m * TLOC + tb * W
                                Yps = psY.tile([128, W], F32, tag="Y")
                                s_lo = 0
                                for gi, gs in enumerate(sizes):
                                    pool = psA if gi % 2 == 0 else psB
                                    cap = 4 if gi % 2 == 0 else 2
                                    Sps = pool.tile([128, cap * W], F32, tag="S")
                                    for jj in range(gs):
                                        st = s_lo + jj
                                        nc.tensor.matmul(
                                            Sps[:, jj * W : (jj + 1) * W],
                                            KT_sb[kbase : kbase + 64, kcol + st * 128 : kcol + (st + 1) * 128],
                                            QT_sb[hbase : hbase + 64, qlo : qlo + W],
                                            start=True,
                                            stop=True,
                                        )
                                    for jj in range(gs):
                                        st = s_lo + jj
                                        if DBG_NO_MASK:
                                            continue
                                        if st >= S_blk - 4:
                                            k = st - (S_blk - 4)
                                            wd = 128 * (k + 1)
                                            nc.vector.tensor_tensor(
                                                Sps[:, jj * W : jj * W + wd],
                                                Sps[:, jj * W : jj * W + wd],
                                                mask_sb[:, 640 - wd : 640],
                                                ADD,
                                            )
                                    PT = work.tile([128, 4 * W], F32R, tag="pt")
                                    nc.scalar.activation(
                                        PT[:, : gs * W], Sps[:, : gs * W], EXPF
                                    )
                                    for jj in range(gs):
                                        st = s_lo + jj
                                        nc.tensor.matmul(
                                            Yps[0:65, :],
                                            V_sb[:, st * 260 + kv * 65 : st * 260 + kv * 65 + 65],
                                            PT[:, jj * W : (jj + 1) * W],
                                            start=(st == 0),
                                            stop=(st == S_blk - 1),
                                        )
                                    s_lo += gs
                                # stage unnormalized Y + l to SBUF; defer
                                # the 1/l chain so it overlaps later heads
                                ysc = work.tile([65, W], F32, tag="ysc", bufs=3)
                                nc.vector.tensor_copy(ysc[:], Yps[0:65, :])

                                def _norm(h=h, tb=tb, ysc=ysc):
                                    lnl = work.tile([1, W], F32, tag="lnl")
                                    nc.scalar.activation(
                                        lnl[:], ysc[64:65, :], LOGF
                                    )
                                    rl = work.tile([1, W], F32R, tag="rl")
                                    nc.scalar.activation(
                                        rl[:], lnl[:], EXPF, scale=-1.0
                                    )
                                    Bps = psN.tile([64, W], F32, tag="bc")
                                    nc.tensor.matmul(
                                        Bps[:],
                                        ones_row[:1, 0:64],
                                        rl[:1, :],
                                        start=True,
                                        stop=True,
                                    )
                                    ct = h // 2
                                    ylo = ct * TLOC + tb * W
                                    if h % 2 == 0:
                                        nc.vector.tensor_tensor(
                                            YT_sb[0:64, ylo : ylo + W],
                                            ysc[0:64, :],
                                            Bps[:],
                                            MULT,
                                        )
                                    else:
                                        osc2 = work.tile([64, W], F32R, tag="osc2")
                                        nc.vector.tensor_tensor(
                                            osc2[:], ysc[0:64, :], Bps[:], MULT
                                        )
                                        nc.sync.dma_start(
                                            out=YT_sb[64:128, ylo : ylo + W],
                                            in_=osc2[:],
                                        )

                                if not DBG_NO_NORM:
                                    norm_queue.append(_norm)
                                if len(norm_queue) > 2:
                                    norm_queue.pop(0)()

                    for fn in norm_queue:
                        fn()
                    norm_queue.clear()

                # ---------------- Phase 4: output projection ----------------
                if upto < 4:
                    continue_p4 = False
                else:
                    continue_p4 = True
                with tc.tile_pool(name="pso", bufs=4, space="PSUM") as pso:
                    for tb in range(NB if continue_p4 else 0):
                        for tt in range(W // 128):
                            stage = work.tile([128, C], F32, tag="ostage")
                            for ob in range(2):
                                ps = pso.tile([128, W], F32, tag="po")
                                for ct in range(8):
                                    ylo = ct * TLOC + tb * W + tt * 128
                                    nc.tensor.matmul(
                                        ps[:],
                                        YT_sb[:, ylo : ylo + 128],
                                        wo_sb[:, ct * C + ob * W : ct * C + (ob + 1) * W],
                                        start=(ct == 0),
                                        stop=False,
                                    )
                                nc.tensor.matmul(
                                    ps[:],
                                    ones_row[:1, 0:128],
                                    bo_row[:1, ob * W : (ob + 1) * W],
                                    start=False,
                                    stop=True,
                                )
                                nc.vector.tensor_copy(
                                    stage[:, ob * W : (ob + 1) * W], ps[:]
                                )
                            nc.sync.dma_start(
                                out=out[tb * W + tt * 128 : tb * W + (tt + 1) * 128, :],
                                in_=stage[:],
                            )
    return _finish(nc, split_waits)


def _finish(nc, split_waits):
    if split_waits:
        split_multi_waits(nc)
    return nc


def host_prep(Wq, bq, Wk, bk, Wv, bv, Wo, bo, qk_gain):
    """Fold gain/sqrt(hd) into Wq/bq, permute q heads, transpose weights."""
    gain = np.asarray(qk_gain, np.float32) / np.float32(np.sqrt(HD))
    gpc = np.repeat(gain, HD)
    Wq_eff = (np.asarray(Wq) * gpc[:, None])[PERM_Q, :]
    bq_eff = (np.asarray(bq) * gpc)[PERM_Q]
    ps = np.arange(128)
    tri = np.where(ps[:, None] <= ps[None, :], 0.0, NEG).astype(np.float32)
    deny = np.full((128, 512), NEG, np.float32)
    maskcat = np.concatenate([deny, tri], axis=1)
    return {
        "wqT": np.ascontiguousarray(Wq_eff.T, np.float32),
        "wkT": np.ascontiguousarray(np.asarray(Wk).T, np.float32),
        "wvT": np.ascontiguousarray(np.asarray(Wv).T, np.float32),
        "woT": np.ascontiguousarray(np.asarray(Wo).T, np.float32),
        "bq_col": np.ascontiguousarray(bq_eff.reshape(8, 128).T, np.float32),
        "bk_col": np.ascontiguousarray(np.asarray(bk).reshape(2, 128).T, np.float32),
        "bv_row": np.asarray(bv, np.float32).reshape(1, -1),
        "bo_row": np.asarray(bo, np.float32).reshape(1, -1),
        "ones_row": np.ones((1, W), np.float32),
        "vones": np.ones((128, 64), np.float32),
        "maskcat": maskcat,
    }


_PROGRAMS = {}


def get_program(t0s):
    key = tuple(t0s)
    if key not in _PROGRAMS:
        _PROGRAMS[key] = build_program(key)
    return _PROGRAMS[key]


def kernel(**inputs):
    x = np.asarray(inputs["x"], np.float32)           # [B, T, C]
    prep = host_prep(
        inputs["Wq"], inputs["bq"], inputs["Wk"], inputs["bk"],
        inputs["Wv"], inputs["bv"], inputs["Wo"], inputs["bo"],
        inputs["qk_gain"],
    )

    from concourse.bass_utils import run_bass_kernel_spmd

    xTs = [np.ascontiguousarray(x[b].T) for b in range(B)]
    ncA = get_program(VAR_A)
    ncB = get_program(VAR_B)
    maps_A = [dict(prep, xT=xTs[b]) for b in range(B)]
    maps_B = [dict(prep, xT=xTs[b]) for b in range(B)]
    resA = run_bass_kernel_spmd(ncA, maps_A, [0, 1, 2, 3]).results
    resB = run_bass_kernel_spmd(ncB, maps_B, [0, 1, 2, 3]).results

    y = np.empty((B, T, C), np.float32)
    for b in range(B):
        oa = resA[b]["out"]
        ob = resB[b]["out"]
        y[b, 0:512] = oa[0:512]
        y[b, 1536:2048] = oa[512:1024]
        y[b, 512:1024] = ob[0:512]
        y[b, 1024:1536] = ob[512:1024]
    return y


# revision 20
# speedup vs baseline: 1.0357x; 1.0357x over previous
"""Causal GQA self-attention (B=4, T=2048, C=1024, H=16, KV=4) on 8 TRN2
NeuronCores via Bass/Tile.

Sharding: core = (batch b, causal-block pair). T is split into four 512-row
blocks k=0..3 with causal cost ~(k+1); pattern A = blocks {0, 3}, pattern
B = {1, 2} — equal work. 8 cores = 4 batches x {A, B}. Two compiled SPMD
programs (one per pattern); no collectives — each core computes its full
output rows locally.

Per core: K/V/Q projections from host-transposed x^T, flash-style attention
in S^T=[keys, queries] layout (f32r matmuls), softmax denominator via an
appended ones-column in V, exp on ScalarE from PSUM, 1/l via exp(-ln(l)) on
ScalarE, broadcast of 1/l across partitions via a K=1 rank-1 matmul, output
projection with biases as K=1 rank-1 matmuls.
"""
import sys

if '/opt/trn_rl_repo' not in sys.path:
    sys.path.insert(0, '/opt/trn_rl_repo')

import numpy as np

# ---------------------------------------------------------------------------
# TileContext tail-drain patch: this walrus rejects instructions carrying
# more than ~2 sync waits; Tile's kernel-tail SP drain accumulates one wait
# per outstanding semaphore. Split the waits onto individual SP nops.
from concourse import tile as _tile


def _patched_drain_and_barrier(self, tick_clock, wait_clock):
    from concourse.tile import ScopedClock

    drain_inst = self.nc.sync.drain()
    wait_clock.add_sem_waits(
        drain_inst.ins, ScopedClock({None: tick_clock.global_clock})
    )
    si = drain_inst.ins.sync_info
    waits = list(si.on_wait) if si and si.on_wait else []
    if len(waits) > 1:
        drain_inst.ins.sync_info = type(si)(on_wait=[waits[0]], on_update=si.on_update)
        for w in waits[1:]:
            nop = self.nc.sync.nop(nofuse=True)
            nop.ins.sync_info = type(si)(on_wait=[w], on_update=[])

    self.nc.all_engine_barrier()
    assert self.sems is not None
    popped = self.nc._tile_sem_poison_stack.pop()
    assert popped is self._sem_poison
    self.nc.clear_and_free_semaphores(list(self.sems.allocated().values()))
    self.nc.all_engine_barrier()


_tile.TileContext._drain_and_barrier = _patched_drain_and_barrier
# ---------------------------------------------------------------------------

_wsplit_counter = [0]


def split_multi_waits(nc, max_waits=1):
    """Walrus in this env rejects instructions with more than ~1-2 embedded
    sync waits. Hoist extra waits onto same-engine NoOps inserted before."""
    import concourse.mybir as mybir

    for f in nc.m.functions:
        for b in f.blocks:
            il = b.instructions
            i = 0
            while i < len(il):
                inst = il[i]
                si = inst.sync_info
                if si is not None and si.on_wait and len(si.on_wait) > max_waits:
                    waits = list(si.on_wait)
                    inst.sync_info = type(si)(
                        on_wait=waits[-max_waits:], on_update=si.on_update or []
                    )
                    for w in waits[: len(waits) - max_waits]:
                        _wsplit_counter[0] += 1
                        nop = mybir.InstNoOp(
                            name=f"wsplit-{_wsplit_counter[0]}", ins=[], outs=[]
                        )
                        nop.engine = inst.engine
                        nop.sync_info = type(si)(on_wait=[w], on_update=[])
                        il.insert(i, nop)
                        i += 1
                i += 1

import concourse.bass as bass
import concourse.mybir as mybir
from concourse import tile

F32 = mybir.dt.float32
F32R = mybir.dt.float32r
EXPF = mybir.ActivationFunctionType.Exp
LOGF = mybir.ActivationFunctionType.Ln
ADD = mybir.AluOpType.add
MULT = mybir.AluOpType.mult
DIV = mybir.AluOpType.divide

B = 4
T = 2048
C = 1024
HD = 64
H = 16
KV = 4
NI = C // 128          # 8 contraction i-tiles
W = 512                # t-block width
NEG = -1e30

# head pairs (even-kv head, odd-kv head) sharing a 128-partition QT chunk
PAIRS = [(0, 4), (1, 5), (2, 6), (3, 7), (8, 12), (9, 13), (10, 14), (11, 15)]
HEADPOS = {}
for _m, (_ha, _hb) in enumerate(PAIRS):
    HEADPOS[_ha] = (_m, 0)
    HEADPOS[_hb] = (_m, 64)
PERM_Q = np.concatenate(
    [np.arange(h * HD, (h + 1) * HD) for pair in PAIRS for h in pair]
)

DBG_NO_MASK = False
DBG_NO_NORM = False
USE_GPSIMD_DIV = False

VAR_A = (0, 1536)
VAR_B = (512, 1024)


def group_plan(S):
    """Split S s-tiles into groups of <=3 (two 3-bank PSUM buffers used in
    global alternation; 1 bank Y accum, 1 bank 1/l broadcast)."""
    sizes = []
    left = S
    while left > 0:
        g = min(3, left)
        sizes.append(g)
        left -= g
    return sizes


def build_program(t0s, T_ctx=T, split_waits=True, upto=4):
    """One SPMD program for cores whose q-blocks start at t0s (globals)."""
    NB = len(t0s)
    TLOC = NB * W
    S_blks = [(t0 + W) // 128 for t0 in t0s]
    NS_KV = max(S_blks)
    NSB4 = (NS_KV * 128 + W - 1) // W

    nc = bass.Bass("TRN2")

    xT = nc.dram_tensor("xT", [C, T_ctx], F32R, kind="ExternalInput")
    wqT = nc.dram_tensor("wqT", [C, C], F32R, kind="ExternalInput")
    wkT = nc.dram_tensor("wkT", [C, KV * HD], F32R, kind="ExternalInput")
    wvT = nc.dram_tensor("wvT", [C, KV * HD], F32R, kind="ExternalInput")
    woT = nc.dram_tensor("woT", [C, C], F32R, kind="ExternalInput")
    bq_col_d = nc.dram_tensor("bq_col", [128, 8], F32, kind="ExternalInput")
    bk_col_d = nc.dram_tensor("bk_col", [128, 2], F32, kind="ExternalInput")
    bv_row_d = nc.dram_tensor("bv_row", [1, KV * HD], F32R, kind="ExternalInput")
    bo_row_d = nc.dram_tensor("bo_row", [1, C], F32R, kind="ExternalInput")
    ones_row_d = nc.dram_tensor("ones_row", [1, W], F32R, kind="ExternalInput")
    vones_d = nc.dram_tensor("vones", [128, 64], F32R, kind="ExternalInput")
    triu_d = nc.dram_tensor("triu", [128, 128], F32R, kind="ExternalInput")
    ident_d = nc.dram_tensor("ident", [128, 128], F32R, kind="ExternalInput")
    indrows_d = nc.dram_tensor("indrows", [1, 3 * W], F32R, kind="ExternalInput")
    out = nc.dram_tensor("out", [TLOC, C], F32, kind="ExternalOutput")

    with tile.TileContext(nc) as tc:
        with (
            tc.tile_pool(name="const", bufs=1) as constp,
            tc.tile_pool(name="big", bufs=1) as big,
        ):
            bq_col = constp.tile([128, 8], F32, tag="bqc")
            bk_col = constp.tile([128, 2], F32, tag="bkc")
            bv_row = constp.tile([1, KV * HD], F32R, tag="bvr")
            bo_row = constp.tile([1, C], F32R, tag="bor")
            ones_row = constp.tile([1, W], F32R, tag="ones")
            triu_sb = constp.tile([128, 128], F32R, tag="triu")
            ident_sb = constp.tile([128, 128], F32R, tag="ident")
            indrows_sb = constp.tile([1, 3 * W], F32R, tag="indr")
            onescol_sb = constp.tile([128, 64], F32R, tag="onescol")
            nc.sync.dma_start(out=bq_col[:], in_=bq_col_d[:])
            nc.sync.dma_start(out=bk_col[:], in_=bk_col_d[:])
            nc.sync.dma_start(out=bv_row[:], in_=bv_row_d[:])
            nc.sync.dma_start(out=bo_row[:], in_=bo_row_d[:])
            nc.sync.dma_start(out=ones_row[:], in_=ones_row_d[:])
            nc.sync.dma_start(out=triu_sb[:], in_=triu_d[:])
            nc.sync.dma_start(out=ident_sb[:], in_=ident_d[:])
            nc.sync.dma_start(out=indrows_sb[:], in_=indrows_d[:])
            nc.sync.dma_start(out=onescol_sb[:], in_=vones_d[:])

            KT_sb = big.tile([128, 2 * T_ctx], F32R, tag="KT")
            V_sb = big.tile([128, 16 * 260], F32R, tag="V")
            QT_sb = big.tile([128, 8 * TLOC], F32R, tag="QT")
            YT_sb = big.tile([128, 8 * TLOC], F32R, tag="YT")

            vview = V_sb[:].rearrange("p (s k e) -> p s k e", s=16, k=KV)
            nc.sync.dma_start(
                out=vview[:, :, :, 64:65],
                in_=vones_d[:].rearrange("p (s k o) -> p s k o", s=16, k=KV),
            )

            # ---------------- Phase 1+2: projections ----------------
            with tc.tile_pool(name="phxt", bufs=1) as phxt:
                xT_sb = phxt.tile([128, NI * T_ctx], F32R, tag="xT")
                for it in range(NI):
                    nc.sync.dma_start(
                        out=xT_sb[:, it * T_ctx : (it + 1) * T_ctx],
                        in_=xT[it * 128 : (it + 1) * 128, :],
                    )

                with (
                    tc.tile_pool(name="phkv", bufs=1) as phkv,
                    tc.tile_pool(name="pskv", bufs=4, space="PSUM") as pskv,
                ):
                    wk_sb = phkv.tile([128, NI * 256], F32R, tag="wk")
                    wv_sb = phkv.tile([128, NI * 256], F32R, tag="wv")
                    for it in range(NI):
                        nc.sync.dma_start(
                            out=wk_sb[:, it * 256 : (it + 1) * 256],
                            in_=wkT[it * 128 : (it + 1) * 128, :],
                        )
                        nc.sync.dma_start(
                            out=wv_sb[:, it * 256 : (it + 1) * 256],
                            in_=wvT[it * 128 : (it + 1) * 128, :],
                        )
                    for j in range(2):
                        for sb in range(NSB4):
                            scols = min(W, NS_KV * 128 - sb * W)
                            ps = pskv.tile([128, W], F32, tag="pk")
                            for it in range(NI):
                                nc.tensor.matmul(
                                    ps[:, :scols],
                                    wk_sb[:, it * 256 + j * 128 : it * 256 + (j + 1) * 128],
                                    xT_sb[:, it * T_ctx + sb * W : it * T_ctx + sb * W + scols],
                                    start=(it == 0),
                                    stop=(it == NI - 1),
                                )
                            nc.vector.tensor_scalar_add(
                                KT_sb[:, j * T_ctx + sb * W : j * T_ctx + sb * W + scols],
                                ps[:, :scols],
                                bk_col[:, j : j + 1],
                            )
                    for st in range(NS_KV):
                        ps = pskv.tile([128, 256], F32, tag="pv")
                        for it in range(NI):
                            nc.tensor.matmul(
                                ps[:],
                                xT_sb[:, it * T_ctx + st * 128 : it * T_ctx + (st + 1) * 128],
                                wv_sb[:, it * 256 : (it + 1) * 256],
                                start=(it == 0),
                                stop=False,
                            )
                        nc.tensor.matmul(
                            ps[:],
                            ones_row[:1, 0:128],
                            bv_row[:1, :],
                            start=False,
                            stop=True,
                        )
                        dst = V_sb[:, st * 260 : st * 260 + 260].rearrange(
                            "p (k e) -> p k e", k=KV
                        )[:, :, 0:64]
                        src = ps[:].rearrange("p (k d) -> p k d", k=KV)
                        nc.vector.tensor_copy(dst, src)

                with (
                    tc.tile_pool(name="phq", bufs=1) as phq,
                    tc.tile_pool(name="psq", bufs=4, space="PSUM") as psq,
                ):
                    wq_sb = phq.tile([128, NI * C], F32R, tag="wq")
                    for it in range(NI):
                        nc.sync.dma_start(
                            out=wq_sb[:, it * C : (it + 1) * C],
                            in_=wqT[it * 128 : (it + 1) * 128, :],
                        )
                    for m in range(8):
                        for tb in range(NB):
                            ps = psq.tile([128, W], F32, tag="pq")
                            for it in range(NI):
                                nc.tensor.matmul(
                                    ps[:],
                                    wq_sb[:, it * C + m * 128 : it * C + (m + 1) * 128],
                                    xT_sb[:, it * T_ctx + t0s[tb] : it * T_ctx + t0s[tb] + W],
                                    start=(it == 0),
                                    stop=(it == NI - 1),
                                )
                            nc.vector.tensor_scalar_add(
                                QT_sb[:, m * TLOC + tb * W : m * TLOC + (tb + 1) * W],
                                ps[:],
                                bq_col[:, m : m + 1],
                            )

            # ---------------- Phase 3: attention ----------------
            if upto < 3:
                return _finish(nc, split_waits)
            with (
                tc.tile_pool(name="ph34", bufs=1) as ph34,
                tc.tile_pool(name="work", bufs=2) as work,
            ):
                wo_sb = ph34.tile([128, NI * C], F32R, tag="wo")
                for ct in range(8):
                    nc.sync.dma_start(
                        out=wo_sb[:, ct * C : (ct + 1) * C],
                        in_=woT[ct * 128 : (ct + 1) * 128, :],
                    )

                with (
                    tc.tile_pool(name="psA", bufs=1, space="PSUM") as psA,
                    tc.tile_pool(name="psB", bufs=1, space="PSUM") as psB,
                    tc.tile_pool(name="psY", bufs=1, space="PSUM") as psY,
                    tc.tile_pool(name="psN", bufs=1, space="PSUM") as psN,
                ):
                    norm_queue = []
                    gparity = [0]
                    for tb in range(NB):
                        S_blk = S_blks[tb]
                        # per s-tile: (k, cut): k = diag index (or -1), cut =
                        # first computed t-col (fully-denied cols skipped for
                        # k >= 2)
                        tinfo = []
                        for st in range(S_blk):
                            diag = (not DBG_NO_MASK) and st >= S_blk - 4
                            k = st - (S_blk - 4) if diag else -1
                            cut = 128 * k if k >= 2 else 0
                            tinfo.append((k, cut))
                        # groups: full-width tiles in 3s; k>=2 tiles alone
                        sizes = []
                        nfull = sum(1 for (k, cut) in tinfo if cut == 0)
                        left = nfull
                        while left > 0:
                            g = min(3, left)
                            sizes.append(g)
                            left -= g
                        for (k, cut) in tinfo:
                            if cut:
                                sizes.append(1)
                        for kv in range(KV):
                            kbase = (kv % 2) * 64
                            kcol = (kv // 2) * T_ctx
                            for g in range(4):
                                h = kv * 4 + g
                                m, hbase = HEADPOS[h]
                                qlo = m * TLOC + tb * W
                                ypool = psY if (kv * 4 + g) % 2 == 0 else psN
                                Yps = ypool.tile([128, W], F32, tag="Y")
                                s_lo = 0
                                for gi, gs in enumerate(sizes):
                                    pool = psA if gparity[0] % 2 == 0 else psB
                                    gparity[0] += 1
                                    Sps = pool.tile([128, 3 * W], F32, tag="S")
                                    g_cut = tinfo[s_lo][1]  # per-tile cut (solo groups)
                                    gw = W - g_cut
                                    for jj in range(gs):
                                        st = s_lo + jj
                                        k, cut = tinfo[st]
                                        nc.tensor.matmul(
                                            Sps[:, jj * gw : jj * gw + gw],
                                            KT_sb[kbase : kbase + 64, kcol + st * 128 : kcol + (st + 1) * 128],
                                            QT_sb[hbase : hbase + 64, qlo + cut : qlo + W],
                                            start=True,
                                            stop=(k < 0),
                                        )
                                        if k >= 0:
                                            # strict-upper -BIG on the diag
                                            # 128-col block via triu^T @ I
                                            nc.tensor.matmul(
                                                Sps[:, jj * gw + 128 * k - cut : jj * gw + 128 * (k + 1) - cut],
                                                triu_sb[:],
                                                ident_sb[:],
                                                start=False,
                                                stop=(k - (cut // 128) == 0),
                                            )
                                            if k > 0 and cut == 0:
                                                # deny cols [0, 128k): rank-1
                                                nc.tensor.matmul(
                                                    Sps[:, jj * gw : jj * gw + 128 * k],
                                                    ones_row[:1, 0:128],
                                                    indrows_sb[0:1, (k - 1) * W : (k - 1) * W + 128 * k],
                                                    start=False,
                                                    stop=True,
                                                )
                                    PT = work.tile([128, 4 * W], F32R, tag="pt")
                                    nc.scalar.activation(
                                        PT[:, : gs * gw], Sps[:, : gs * gw], EXPF
                                    )
                                    for jj in range(gs):
                                        st = s_lo + jj
                                        k, cut = tinfo[st]
                                        nc.tensor.matmul(
                                            Yps[0:65, cut:W],
                                            V_sb[:, st * 260 + kv * 65 : st * 260 + kv * 65 + 65],
                                            PT[:, jj * gw : jj * gw + gw],
                                            start=(st == 0),
                                            stop=(st == S_blk - 1),
                                        )
                                    s_lo += gs
                                # stage unnormalized Y + l to SBUF; defer
                                # the 1/l chain so it overlaps later heads
                                ysc = work.tile([65, W], F32R, tag="ysc", bufs=3)
                                nc.vector.tensor_copy(ysc[:], Yps[0:65, :])

                                def _norm(h=h, tb=tb, ysc=ysc, ypool=ypool):
                                    ct = h // 2
                                    ylo = ct * TLOC + tb * W
                                    if USE_GPSIMD_DIV:
                                        # broadcast raw l; divide on GpSimd
                                        Bps = ypool.tile([64, W], F32, tag="Y")
                                        nc.tensor.matmul(
                                            Bps[:],
                                            onescol_sb[64:65, :],
                                            ysc[64:65, :],
                                            start=True,
                                            stop=True,
                                        )
                                        lb = work.tile([64, W], F32, tag="lb")
                                        nc.vector.tensor_copy(lb[:], Bps[:])
                                        if h % 2 == 0:
                                            nc.gpsimd.tensor_tensor(
                                                YT_sb[0:64, ylo : ylo + W],
                                                ysc[0:64, :],
                                                lb[:],
                                                DIV,
                                            )
                                        else:
                                            osc2 = work.tile([64, W], F32R, tag="osc2")
                                            nc.gpsimd.tensor_tensor(
                                                osc2[:], ysc[0:64, :], lb[:], DIV
                                            )
                                            nc.sync.dma_start(
                                                out=YT_sb[64:128, ylo : ylo + W],
                                                in_=osc2[:],
                                            )
                                        return
                                    lnl = work.tile([1, W], F32, tag="lnl")
                                    nc.scalar.activation(
                                        lnl[:], ysc[64:65, :], LOGF
                                    )
                                    rl = work.tile([1, W], F32R, tag="rl")
                                    nc.scalar.activation(
                                        rl[:], lnl[:], EXPF, scale=-1.0
                                    )
                                    Bps = ypool.tile([64, W], F32, tag="Y")
                                    nc.tensor.matmul(
                                        Bps[:],
                                        ones_row[:1, 0:64],
                                        rl[:1, :],
                                        start=True,
                                        stop=True,
                                    )
                                    if h % 2 == 0:
                                        nc.vector.tensor_tensor(
                                            YT_sb[0:64, ylo : ylo + W],
                                            ysc[0:64, :],
                                            Bps[:],
                                            MULT,
                                        )
                                    else:
                                        osc2 = work.tile([64, W], F32R, tag="osc2")
                                        nc.vector.tensor_tensor(
                                            osc2[:], ysc[0:64, :], Bps[:], MULT
                                        )
                                        nc.sync.dma_start(
                                            out=YT_sb[64:128, ylo : ylo + W],
                                            in_=osc2[:],
                                        )

                                if DBG_NO_NORM:
                                    ct0 = h // 2
                                    ylo0 = ct0 * TLOC + tb * W
                                    if h % 2 == 0:
                                        nc.vector.tensor_copy(
                                            YT_sb[0:64, ylo0 : ylo0 + W],
                                            ysc[0:64, :],
                                        )
                                    else:
                                        osc3 = work.tile([64, W], F32R, tag="osc2")
                                        nc.vector.tensor_copy(osc3[:], ysc[0:64, :])
                                        nc.sync.dma_start(
                                            out=YT_sb[64:128, ylo0 : ylo0 + W],
                                            in_=osc3[:],
                                        )
                                else:
                                    norm_queue.append(_norm)
                                if len(norm_queue) > 2:
                                    norm_queue.pop(0)()

                    for fn in norm_queue:
                        fn()
                    norm_queue.clear()

                # ---------------- Phase 4: output projection ----------------
                if upto < 4:
                    continue_p4 = False
                else:
                    continue_p4 = True
                with tc.tile_pool(name="pso", bufs=4, space="PSUM") as pso:
                    for tb in range(NB if continue_p4 else 0):
                        for tt in range(W // 128):
                            stage = work.tile([128, C], F32, tag="ostage")
                            for ob in range(2):
                                ps = pso.tile([128, W], F32, tag="po")
                                for ct in range(8):
                                    ylo = ct * TLOC + tb * W + tt * 128
                                    nc.tensor.matmul(
                                        ps[:],
                                        YT_sb[:, ylo : ylo + 128],
                                        wo_sb[:, ct * C + ob * W : ct * C + (ob + 1) * W],
                                        start=(ct == 0),
                                        stop=False,
                                    )
                                nc.tensor.matmul(
                                    ps[:],
                                    ones_row[:1, 0:128],
                                    bo_row[:1, ob * W : (ob + 1) * W],
                                    start=False,
                                    stop=True,
                                )
                                nc.vector.tensor_copy(
                                    stage[:, ob * W : (ob + 1) * W], ps[:]
                                )
                            nc.sync.dma_start(
                                out=out[tb * W + tt * 128 : tb * W + (tt + 1) * 128, :],
                                in_=stage[:],
                            )
    return _finish(nc, split_waits)


def _finish(nc, split_waits):
    if split_waits:
        split_multi_waits(nc)
    return nc


def host_prep(Wq, bq, Wk, bk, Wv, bv, Wo, bo, qk_gain):
    """Fold gain/sqrt(hd) into Wq/bq, permute q heads, transpose weights."""
    gain = np.asarray(qk_gain, np.float32) / np.float32(np.sqrt(HD))
    gpc = np.repeat(gain, HD)
    Wq_eff = (np.asarray(Wq) * gpc[:, None])[PERM_Q, :]
    bq_eff = (np.asarray(bq) * gpc)[PERM_Q]
    return {
        "wqT": np.ascontiguousarray(Wq_eff.T, np.float32),
        "wkT": np.ascontiguousarray(np.asarray(Wk).T, np.float32),
        "wvT": np.ascontiguousarray(np.asarray(Wv).T, np.float32),
        "woT": np.ascontiguousarray(np.asarray(Wo).T, np.float32),
        "bq_col": np.ascontiguousarray(bq_eff.reshape(8, 128).T, np.float32),
        "bk_col": np.ascontiguousarray(np.asarray(bk).reshape(2, 128).T, np.float32),
        "bv_row": np.asarray(bv, np.float32).reshape(1, -1),
        "bo_row": np.asarray(bo, np.float32).reshape(1, -1),
        "ones_row": np.ones((1, W), np.float32),
        "vones": np.ones((128, 64), np.float32),
        "triu": np.where(
            np.arange(128)[:, None] < np.arange(128)[None, :], NEG, 0.0
        ).astype(np.float32),
        "ident": np.eye(128, dtype=np.float32),
        "indrows": np.concatenate(
            [
                np.where(np.arange(W) < 128 * (j + 1), NEG, 0.0)
                for j in range(3)
            ]
        ).astype(np.float32).reshape(1, -1),
    }


_PROGRAMS = {}


def get_program(t0s):
    key = tuple(t0s)
    if key not in _PROGRAMS:
        _PROGRAMS[key] = build_program(key)
    return _PROGRAMS[key]


def kernel(**inputs):
    x = np.asarray(inputs["x"], np.float32)           # [B, T, C]
    prep = host_prep(
        inputs["Wq"], inputs["bq"], inputs["Wk"], inputs["bk"],
        inputs["Wv"], inputs["bv"], inputs["Wo"], inputs["bo"],
        inputs["qk_gain"],
    )

    from concourse.bass_utils import run_bass_kernel_spmd

    xTs = [np.ascontiguousarray(x[b].T) for b in range(B)]
    ncA = get_program(VAR_A)
    ncB = get_program(VAR_B)
    maps_A = [dict(prep, xT=xTs[b]) for b in range(B)]
    maps_B = [dict(prep, xT=xTs[b]) for b in range(B)]
    resA = run_bass_kernel_spmd(ncA, maps_A, [0, 1, 2, 3]).results
    resB = run_bass_kernel_spmd(ncB, maps_B, [0, 1, 2, 3]).results

    y = np.empty((B, T, C), np.float32)
    for b in range(B):
        oa = resA[b]["out"]
        ob = resB[b]["out"]
        y[b, 0:512] = oa[0:512]
        y[b, 1536:2048] = oa[512:1024]
        y[b, 512:1024] = ob[0:512]
        y[b, 1024:1536] = ob[512:1024]
    return y
